# revision 1
# baseline (speedup 1.0000x reference)
"""LID detector kernel for Trainium2 (8 NeuronCores, data-parallel over batch).

Per core (batch shard of 32):
  - mean-pool each feature map over space -> q [C, 32] (transposed layout)
  - -d2 = 2*q.r - ||r||^2 - ||q||^2 via PE matmuls into PSUM, evicted into a
    stacked [128, 2000] buffer (partition quadrant = layer)
  - top-24 smallest d2 via 3 rounds of DVE max8 + match_replace
  - LID = -2k / (sum_{i=1..20} ln d2_i - 20 ln d2_20)  (no sqrt needed)
  - logit = w . lid + b -> sigmoid -> out [32]
"""

import sys

for _p in ("/opt/trn_rl_repo", "/root/.axon_site/_ro/trn_rl_repo"):
    if _p not in sys.path:
        sys.path.append(_p)

import ml_dtypes
import numpy as np

import concourse.mybir as mybir
from concourse import bass, bacc
from concourse.tile import TileContext
from concourse.bass_utils import run_bass_kernel_spmd

F32 = mybir.dt.float32
BF16 = mybir.dt.bfloat16
N_CORES = 8
B = 32  # batch shard per core
R = 2000
K = 20
LAYERS = [(64, 3136), (128, 784), (256, 196), (512, 49)]  # (C, H*W)
NEG_BIG = -3.0e38

# column j of qT holds sample SIGMA[j] of the local shard
SIGMA = np.array([2 * j for j in range(16)] + [2 * j + 1 for j in range(16)])


def build_nc():
    nc = bacc.Bacc("TRN2", target_bir_lowering=False, debug=False,
                   num_devices=N_CORES)

    feats = [nc.dram_tensor(f"feat{l}", [B, C, HW], BF16, kind="ExternalInput")
             for l, (C, HW) in enumerate(LAYERS)]
    refTs = [nc.dram_tensor(f"refT{l}", [C, R], F32, kind="ExternalInput")
             for l, (C, _) in enumerate(LAYERS)]
    regw = nc.dram_tensor("regw", [1, 4], F32, kind="ExternalInput")
    regb = nc.dram_tensor("regb", [1, 1], F32, kind="ExternalInput")
    out = nc.dram_tensor("out", [B, 1], F32, kind="ExternalOutput")
    import os
    _dbg = os.environ.get("DEBUG_LID") == "1"
    if _dbg:
        dbg_lid = nc.dram_tensor("dbg_lid", [128, 1], F32, kind="ExternalOutput")
        dbg_vals = nc.dram_tensor("dbg_vals", [128, 24], F32, kind="ExternalOutput")
        dbg_q = nc.dram_tensor("dbg_q", [64, B], F32, kind="ExternalOutput")
        dbg_tk = nc.dram_tensor("dbg_tk", [128, R], F32, kind="ExternalOutput")
        dbg_rn2a = nc.dram_tensor("dbg_rn2a", [65, R], F32, kind="ExternalOutput")
        dbg_rn2b = nc.dram_tensor("dbg_rn2b", [1, R], F32, kind="ExternalOutput")

    with TileContext(nc) as tc:
        with (
            tc.tile_pool(name="persist", bufs=1) as pp,
            tc.tile_pool(name="ft", bufs=6) as fp,
            tc.tile_pool(name="sq", bufs=2) as sqp,
        ):
            # ---- persistent tiles
            rt = {}   # (l, i) -> refT chunk tile [Cc, R]
            for l, (C, _) in enumerate(LAYERS):
                for i in range(0, C, 128):
                    Cc = min(128, C - i)
                    rt[(l, i)] = pp.tile([Cc, R], F32, tag=f"rt{l}_{i}",
                                         name=f"rt{l}_{i}")
            act_scratch = pp.tile([128, 3136], BF16, tag="act_scratch",
                                  name="act_scratch")
            rn2a = pp.tile([65, R], F32, tag="rn2a", name="rn2a")
            rn2b = pp.tile([1, R], F32, tag="rn2b", name="rn2b")
            rn2base = [(rn2a, 0), (rn2a, 32), (rn2a, 64), (rn2b, 0)]
            rn2row = [t[b:b + 1, :] for (t, b) in rn2base]
            qT = {}
            for l, (C, _) in enumerate(LAYERS):
                for i in range(0, C, 128):
                    Cc = min(128, C - i)
                    qT[(l, i)] = pp.tile([Cc, B], F32, tag=f"qT{l}_{i}", name=f"qT{l}_{i}")
            qn2neg = [pp.tile([B, 1], F32, tag=f"qn2_{l}", name=f"qn2_{l}") for l in range(4)]
            topkbuf = pp.tile([128, R], F32, tag="topkbuf", name="topkbuf")
            vals = pp.tile([128, 24], F32, tag="vals", name="vals")
            ones_col = pp.tile([128, 1], F32, tag="ones_col", name="ones_col")
            ones_row = pp.tile([1, B], F32, tag="ones_row", name="ones_row")
            negones_all = pp.tile([65, B], F32, tag="negones_all", name="negones_all")
            wb_sb = pp.tile([1, 5], F32, tag="wb_sb", name="wb_sb")
            tmp0 = pp.tile([128, 16], F32, tag="tmp0", name="tmp0")

            nc.vector.memset(ones_col[:], 1.0)
            nc.vector.memset(ones_row[:], 1.0)
            nc.vector.memset(negones_all[:], -1.0)
            nc.sync.dma_start(out=wb_sb[0:1, 0:4], in_=regw[:])
            nc.sync.dma_start(out=wb_sb[0:1, 4:5], in_=regb[:])

            # ---- ref squared norms: rn2[l] [1, R] = sum_c refT^2
            with tc.tile_pool(name="psumA", bufs=1,
                              space=bass.MemorySpace.PSUM) as pA:
                for l, (C, _) in enumerate(LAYERS):
                    ps = pA.tile([1, R], F32, tag="rn2ps", name="rn2ps")
                    chunks = list(range(0, C, 128))
                    for ci, i in enumerate(chunks):
                        Cc = min(128, C - i)
                        sq = sqp.tile([128, R], F32, tag="sq", name="sq")
                        nc.scalar.square(sq[0:Cc, :], rt[(l, i)][:])
                        for c0 in range(0, R, 512):
                            n = min(512, R - c0)
                            nc.tensor.matmul(
                                ps[0:1, c0:c0 + n],
                                ones_col[0:Cc, 0:1],
                                sq[0:Cc, c0:c0 + n],
                                start=(ci == 0), stop=(ci == len(chunks) - 1),
                            )
                    nc.scalar.copy(rn2row[l], ps[:])

            # ---- pooling: fill qT columns (sample order SIGMA)
            # layer 0: C=64, 2 samples per 128 partitions
            C, HW = LAYERS[0]
            for t in range(8):
                tile = fp.tile([128, 2, HW], BF16, tag="ft", name="ft")
                src = bass.AP(feats[0], 4 * t * C * HW,
                              [[HW, 128], [2 * C * HW, 2], [1, HW]])
                nc.sync.dma_start(out=tile[:], in_=src)
                if t < 4:
                    nc.vector.tensor_reduce(
                        tmp0[:, 2 * t:2 * t + 2], tile[:],
                        axis=mybir.AxisListType.X, op=mybir.AluOpType.add)
                else:
                    for g in range(2):
                        nc.scalar.activation(
                            act_scratch[:, 0:HW], tile[:, g, :],
                            mybir.ActivationFunctionType.Copy,
                            accum_out=tmp0[:, 2 * t + g:2 * t + g + 1])
            nc.vector.tensor_copy(qT[(0, 0)][:, 0:16], tmp0[0:64, :])
            nc.vector.tensor_copy(qT[(0, 0)][:, 16:32], tmp0[64:128, :])

            # layers 1..3: per chunk, DMA samples with stride 2 (even then odd)
            for l in (1, 2, 3):
                C, HW = LAYERS[l]
                n_chunks = C // 128
                g = B // (2 * 4 // 1)  # placeholder, set below
                # samples per DMA: L1:4 (8 DMAs), L2:8 (4 DMAs), L3:16 (2 DMAs)
                spd = {1: 4, 2: 8, 3: 16}[l]
                ndma = B // spd
                for i in range(n_chunks):
                    for t in range(ndma):
                        # cols spd*t .. spd*t+spd-1 -> samples SIGMA[col]
                        # = base + 2*j, base = 2*spd*t if even half else ...
                        col0 = spd * t
                        s_base = int(SIGMA[col0])
                        tile = fp.tile([128, spd, HW], BF16, tag="ft", name="ft")
                        src = bass.AP(
                            feats[l],
                            s_base * C * HW + 128 * i * HW,
                            [[HW, 128], [2 * C * HW, spd], [1, HW]])
                        nc.sync.dma_start(out=tile[:], in_=src)
                        if l == 1:
                            for g in range(spd):
                                nc.scalar.activation(
                                    act_scratch[:, 0:HW], tile[:, g, :],
                                    mybir.ActivationFunctionType.Copy,
                                    accum_out=qT[(l, 128 * i)][:, col0 + g:col0 + g + 1])
                        else:
                            nc.vector.tensor_reduce(
                                qT[(l, 128 * i)][:, col0:col0 + spd], tile[:],
                                axis=mybir.AxisListType.X, op=mybir.AluOpType.add)

            for l, (C, _) in enumerate(LAYERS):
                for i in range(0, C, 128):
                    Cc = min(128, C - i)
                    nc.sync.dma_start(out=rt[(l, i)][:],
                                      in_=refTs[l][i:i + Cc, :])

            # ---- scale qT by 2/HW (so lhsT holds 2*q), qn2neg
            with tc.tile_pool(name="psumB", bufs=1,
                              space=bass.MemorySpace.PSUM) as pB:
                for l, (C, HW) in enumerate(LAYERS):
                    chunks = list(range(0, C, 128))
                    qps = pB.tile([B, 1], F32, tag="qn2ps", name="qn2ps")
                    for ci, i in enumerate(chunks):
                        Cc = min(128, C - i)
                        nc.scalar.mul(qT[(l, i)][:], qT[(l, i)][:], 2.0 / HW)
                        qsq = sqp.tile([128, B], F32, tag="qsq", name="qsq")
                        # (2q * 0.5)^2 = q^2
                        nc.scalar.activation(
                            qsq[0:Cc, :], qT[(l, i)][:],
                            mybir.ActivationFunctionType.Square, scale=0.5)
                        nc.tensor.matmul(
                            qps[:], qsq[0:Cc, :], ones_col[0:Cc, 0:1],
                            start=(ci == 0), stop=(ci == len(chunks) - 1))
                    nc.scalar.mul(qn2neg[l][:], qps[:], -1.0)

                # ---- distances: psum = 2q.r - rn2 ; evict + qn2neg -> -d2
                for l, (C, _) in enumerate(LAYERS):
                    chunks = list(range(0, C, 128))
                    for c0 in range(0, R, 512):
                        n = min(512, R - c0)
                        dps = pB.tile([B, 512], F32, tag="d2ps", name="d2ps")
                        for ci, i in enumerate(chunks):
                            Cc = min(128, C - i)
                            nc.tensor.matmul(
                                dps[:, 0:n], qT[(l, i)][:],
                                rt[(l, i)][:, c0:c0 + n],
                                start=(ci == 0), stop=False)
                        rn2t, rn2b_ = rn2base[l]
                        nc.tensor.matmul(
                            dps[:, 0:n], negones_all[rn2b_:rn2b_ + 1, :],
                            rn2t[rn2b_:rn2b_ + 1, c0:c0 + n],
                            start=False, stop=True)
                        nc.vector.tensor_scalar(
                            topkbuf[32 * l:32 * l + 32, c0:c0 + n],
                            dps[:, 0:n], qn2neg[l][:], None,
                            op0=mybir.AluOpType.add)

                if _dbg:
                    nc.sync.dma_start(out=dbg_tk[:], in_=topkbuf[:])
                    nc.sync.dma_start(out=dbg_rn2a[:], in_=rn2a[:])
                    nc.sync.dma_start(out=dbg_rn2b[:], in_=rn2b[:])
                # ---- top-24 (ascending d2 == descending -d2)
                nc.vector.max(vals[:, 0:8], topkbuf[:])
                nc.vector.match_replace(topkbuf[:], vals[:, 0:8], topkbuf[:],
                                        NEG_BIG)
                nc.vector.max(vals[:, 8:16], topkbuf[:])
                nc.vector.match_replace(topkbuf[:], vals[:, 8:16], topkbuf[:],
                                        NEG_BIG)
                nc.vector.max(vals[:, 16:24], topkbuf[:])

                # ---- LID
                ln2 = pp.tile([128, 24], F32, tag="ln2", name="ln2")
                S = pp.tile([128, 1], F32, tag="S", name="S")
                denom = pp.tile([128, 1], F32, tag="denom", name="denom")
                lid = pp.tile([128, 1], F32, tag="lid", name="lid")
                # clamp: vals <= -1e-30 so that -vals >= 1e-30
                nc.vector.tensor_scalar_min(vals[:], vals[:], -1e-30)
                nc.scalar.activation(ln2[:], vals[:],
                                     mybir.ActivationFunctionType.Ln,
                                     scale=-1.0)
                nc.vector.tensor_reduce(S[:], ln2[:, 1:21],
                                        axis=mybir.AxisListType.X,
                                        op=mybir.AluOpType.add)
                # denom = -20*ln2[20] + S  (= sum ln d2_i - 20 ln d2_20)
                nc.vector.tensor_scalar(denom[:], ln2[:, 20:21], -20.0, S[:],
                                        op0=mybir.AluOpType.mult,
                                        op1=mybir.AluOpType.add)
                nc.vector.reciprocal(lid[:], denom[:])
                nc.vector.tensor_scalar_mul(lid[:], lid[:], -2.0 * K)
                if _dbg:
                    nc.sync.dma_start(out=dbg_lid[:], in_=lid[:])
                    nc.sync.dma_start(out=dbg_vals[:], in_=vals[:])
                    nc.sync.dma_start(out=dbg_q[:], in_=qT[(0, 0)][:])

                # ---- regression + sigmoid
                lid4 = pp.tile([B, 4], F32, tag="lid4", name="lid4")
                for l in range(4):
                    nc.vector.tensor_copy(lid4[:, l:l + 1],
                                          lid[32 * l:32 * l + 32, :])
                wps = pB.tile([B, 5], F32, tag="wps", name="wps")
                nc.tensor.matmul(wps[:], ones_row[:], wb_sb[:],
                                 start=True, stop=True)
                wbc = pp.tile([B, 5], F32, tag="wbc", name="wbc")
                nc.scalar.copy(wbc[:], wps[:])
                prod = pp.tile([B, 4], F32, tag="prod", name="prod")
                nc.vector.tensor_tensor(prod[:], lid4[:], wbc[:, 0:4],
                                        op=mybir.AluOpType.mult)
                ssum = pp.tile([B, 1], F32, tag="ssum", name="ssum")
                nc.vector.tensor_reduce(ssum[:], prod[:],
                                        axis=mybir.AxisListType.X,
                                        op=mybir.AluOpType.add)
                res = pp.tile([B, 1], F32, tag="res", name="res")
                nc.scalar.activation(res[:], ssum[:],
                                     mybir.ActivationFunctionType.Sigmoid,
                                     bias=wbc[:, 4:5])
                nc.sync.dma_start(out=out[:], in_=res[:])

    nc.compile()
    return nc


_NC = None


def _get_nc():
    global _NC
    if _NC is None:
        _NC = build_nc()
    return _NC


def run(trace=False, **inputs):
    nc = _get_nc()
    feats = [np.asarray(inputs[f"feat{l}"], dtype=np.float32) for l in range(4)]
    refTs = [np.ascontiguousarray(np.asarray(inputs[f"ref{l}"],
                                             dtype=np.float32).T)
             for l in range(4)]
    regw = np.asarray(inputs["reg_w"], dtype=np.float32).reshape(1, 4)
    regb = np.asarray(inputs["reg_b"], dtype=np.float32).reshape(1, 1)
    assert int(inputs.get("k", K)) == K

    in_maps = []
    for c in range(N_CORES):
        m = {}
        for l, (C, HW) in enumerate(LAYERS):
            m[f"feat{l}"] = np.ascontiguousarray(
                feats[l][c * B:(c + 1) * B].reshape(B, C, HW)).astype(
                    ml_dtypes.bfloat16)
            m[f"refT{l}"] = refTs[l]
        m["regw"] = regw
        m["regb"] = regb
        in_maps.append(m)

    res = run_bass_kernel_spmd(nc, in_maps, core_ids=list(range(N_CORES)),
                               trace=trace)
    full = np.empty((N_CORES * B,), dtype=np.float32)
    for c in range(N_CORES):
        shard = np.empty((B,), dtype=np.float32)
        shard[SIGMA] = res.results[c]["out"][:, 0]
        full[c * B:(c + 1) * B] = shard
    return full, res


def kernel(**inputs):
    return run(trace=False, **inputs)[0]



# revision 5
# speedup vs baseline: 1.3927x; 1.3927x over previous
"""LID detector kernel for Trainium2 (8 NeuronCores, data-parallel over batch).

Per core (batch shard of 32 samples):
  - features arrive as fp8(e4m3); mean-pool over space on DVE/ACT/GPSIMD
    into f32 sums, scaled into fp16 qT tiles (holding 2*q)
  - refs arrive as fp16 [C, R] (host-transposed; L2/L3 rows permuted to match
    the on-chip channel interleave — distances are invariant to a shared
    channel permutation of q and ref)
  - big[row,(l,s), ref] = 2q.r - ||r||^2 accumulated by fp16 PE matmuls into
    one [128, 2000] PSUM tile: dist matmuls (lhsT=qT) plus fold matmuls
    (lhsT=-1s, rhs=sq=rt^2).  L2+L3 share the base-64 quadrant with
    zero-padded lhsT column blocks (matmul PSUM base partition must be
    0/32/64).
  - top-24 via DVE max8/match_replace straight on PSUM, split into two
    column halves + a 48-wide merge so half A overlaps half B's matmuls
  - vals += -qn2 per row -> -d2; LID = -2k/(sum ln d2_i - 20 ln d2_20);
    4->1 regression; sigmoid
"""

import sys

for _p in ("/opt/trn_rl_repo", "/root/.axon_site/_ro/trn_rl_repo"):
    if _p not in sys.path:
        sys.path.append(_p)

import ml_dtypes
import numpy as np

import concourse.mybir as mybir
from concourse import bass, bacc
from concourse.tile import TileContext
from concourse.bass_utils import run_bass_kernel_spmd

F32 = mybir.dt.float32
F16 = mybir.dt.float16
F8 = mybir.dt.float8e4
N_CORES = 8
B = 32
R = 2000
K = 20
NEG_BIG = -3.0e38
ALU = mybir.AluOpType
ACTF = mybir.ActivationFunctionType
AX = mybir.AxisListType

# (C, HW, channels-per-partition)
LAYERS = [(64, 3136, 1), (128, 784, 1), (256, 196, 2), (512, 49, 4)]

# --- engine assignment knobs -----------------------------------------------
# L0: 16 pairs (2 samples x 64ch on 128 partitions, 3136 free cols each).
#   'A' ACT accum | 'V' DVE reduce | 'GV'/'GA' gpsimd halve + DVE/ACT tail
L0_ASSIGN = ["GV", "A", "GV", "A", "GV", "A", "GV", "A",
             "GV", "A", "GV", "A", "GV", "A", "GV", "A"]
# L1: 8 tiles [128, 4, 784]. 'V' | 'A' (4 ops) | 'GV'
L1_ASSIGN = ["GV", "A", "V", "GV", "A", "V", "GV", "A"]
# L2: 8 tiles [128, 4, 2, 196]. 'V' | 'GV'
L2_ASSIGN = ["GV", "V", "GV", "V", "GV", "V", "GV", "V"]
# L3: 8 tiles [128, 4, 4, 49]. 'V' | 'GV'
L3_ASSIGN = ["V", "V", "V", "V", "V", "V", "V", "V"]
# squares, keyed (layer, chunk_offset): 'A' | 'V' | 'G'
SQ_ASSIGN = {(0, 0): "A", (1, 0): "A", (2, 0): "A", (2, 128): "A",
             (3, 0): "G", (3, 128): "G", (3, 256): "A", (3, 384): "A"}

RCHUNKS = [(0, 512), (512, 512), (1024, 512), (1536, 464)]
HALF_A = (0, 1024)
HALF_B = (1024, 976)


def build_nc():
    nc = bacc.Bacc("TRN2", target_bir_lowering=False, debug=False,
                   num_devices=N_CORES)

    feats = [nc.dram_tensor(f"feat{l}", [B, C, HW], F8, kind="ExternalInput")
             for l, (C, HW, _) in enumerate(LAYERS)]
    refTs = [nc.dram_tensor(f"refT{l}", [C, R], F16, kind="ExternalInput")
             for l, (C, _, _) in enumerate(LAYERS)]
    regw = nc.dram_tensor("regw", [1, 4], F32, kind="ExternalInput")
    regb = nc.dram_tensor("regb", [1, 1], F32, kind="ExternalInput")
    out = nc.dram_tensor("out", [B, 1], F32, kind="ExternalOutput")

    with TileContext(nc) as tc:
        with (
            tc.tile_pool(name="persist", bufs=1) as pp,
            tc.tile_pool(name="ft0", bufs=3) as fp0,
            tc.tile_pool(name="ft1", bufs=3) as fp1,
            tc.tile_pool(name="ft23", bufs=3) as fp23,
            tc.tile_pool(name="half", bufs=3) as hvp,
            tc.tile_pool(name="pbig", bufs=1,
                         space=bass.MemorySpace.PSUM) as pbig,
            tc.tile_pool(name="psmall", bufs=1,
                         space=bass.MemorySpace.PSUM) as psml,
        ):
            # ---------------- persistent tiles
            rt = {}
            sq = {}
            for l, (C, _, _) in enumerate(LAYERS):
                for i in range(0, C, 128):
                    Cc = min(128, C - i)
                    rt[(l, i)] = pp.tile([Cc, R], F16, tag=f"rt{l}_{i}",
                                         name=f"rt{l}_{i}")
                    sq[(l, i)] = pp.tile([Cc, R], F16, tag=f"sq{l}_{i}",
                                         name=f"sq{l}_{i}")
            sums0 = pp.tile([128, 16], F32, tag="sums0", name="sums0")
            sums1 = pp.tile([128, B], F32, tag="sums1", name="sums1")
            sums2 = pp.tile([128, B, 2], F32, tag="sums2", name="sums2")
            sums3 = pp.tile([128, B, 4], F32, tag="sums3", name="sums3")
            qT0 = pp.tile([64, 16, 2], F16, tag="qT0", name="qT0")
            qT1 = pp.tile([128, B], F16, tag="qT1", name="qT1")
            qT23 = {}
            for l, nch in ((2, 2), (3, 4)):
                for j in range(nch):
                    qT23[(l, j)] = pp.tile([128, 64], F16, tag=f"qT{l}_{j}",
                                           name=f"qT{l}_{j}")
            ones_col = pp.tile([128, 1], F16, tag="ones_col", name="ones_col")
            ones_row = pp.tile([1, B], F32, tag="ones_row", name="ones_row")
            # fold lhsT masks: row usage via column slices
            neg_full = pp.tile([128, 64], F16, tag="neg_full", name="neg_full")
            neg_l2 = pp.tile([128, 64], F16, tag="neg_l2", name="neg_l2")
            neg_l3 = pp.tile([128, 64], F16, tag="neg_l3", name="neg_l3")
            wb_sb = pp.tile([1, 5], F32, tag="wb_sb", name="wb_sb")
            vals48 = pp.tile([128, 48], F32, tag="vals48", name="vals48")
            vals = pp.tile([128, 24], F32, tag="vals", name="vals")
            fixrow = pp.tile([128, 1], F32, tag="fixrow", name="fixrow")
            scr0 = pp.tile([128, 3136], F8, tag="scr0", name="scr0")
            scr0b = pp.tile([128, 1568], F16, tag="scr0b", name="scr0b")
            scr1 = pp.tile([128, 784], F8, tag="scr1", name="scr1")

            nc.vector.memset(ones_col[:], 1.0)
            nc.vector.memset(ones_row[:], 1.0)
            nc.vector.memset(neg_full[:], -1.0)
            nc.vector.memset(neg_l2[:, 0:32], -1.0)
            nc.vector.memset(neg_l2[:, 32:64], 0.0)
            nc.vector.memset(neg_l3[:, 0:32], 0.0)
            nc.vector.memset(neg_l3[:, 32:64], -1.0)
            for l, nch in ((2, 2), (3, 4)):
                zs = slice(32, 64) if l == 2 else slice(0, 32)
                for j in range(nch):
                    nc.vector.memset(qT23[(l, j)][:, zs], 0.0)
            nc.sync.dma_start(out=wb_sb[0:1, 0:4], in_=regw[:])
            nc.sync.dma_start(out=wb_sb[0:1, 4:5], in_=regb[:])

            # ---------------- DMAs: interleave L0 feature tiles with refs
            ref_list = [(l, i) for l, (C, _, _) in enumerate(LAYERS)
                        for i in range(0, C, 128)]
            ftiles = {}

            def dma_feat(l, t):
                C, HW, cpp = LAYERS[l]
                nS = 2 if l == 0 else 4
                if l == 0:
                    tile = fp0.tile([128, nS, HW], F8, tag="f0",
                                    name=f"f0_{t}")
                    src = bass.AP(feats[0], 4 * t * C * HW,
                                  [[HW, 128], [2 * C * HW, nS], [1, HW]])
                else:
                    fpool = fp1 if l == 1 else fp23
                    tile = fpool.tile([128, nS, cpp, HW], F8, tag=f"f{l}",
                                      name=f"f{l}_{t}")
                    src = bass.AP(feats[l], nS * t * C * HW,
                                  [[cpp * HW, 128], [C * HW, nS],
                                   [HW, cpp], [1, HW]])
                nc.sync.dma_start(out=tile[:], in_=src)
                ftiles[(l, t)] = tile

            for t in range(8):
                dma_feat(0, t)
                li = ref_list[t]
                nc.sync.dma_start(out=rt[li][:],
                                  in_=refTs[li[0]][li[1]:li[1] + rt[li].shape[0], :])
            for t in range(8):
                dma_feat(1, t)
            for t in range(8):
                dma_feat(2, t)
            for t in range(8):
                dma_feat(3, t)

            # ---------------- squares (sq = rt^2, fp16)
            for (l, i), eng in SQ_ASSIGN.items():
                s_, r_ = sq[(l, i)], rt[(l, i)]
                if eng == "A":
                    nc.scalar.square(s_[:], r_[:])
                elif eng == "V":
                    nc.vector.tensor_tensor(s_[:], r_[:], r_[:], op=ALU.mult)
                else:
                    nc.gpsimd.tensor_tensor(s_[:], r_[:], r_[:], op=ALU.mult)

            # ---------------- pooling
            # L0: pairs
            C, HW, _ = LAYERS[0]
            for t in range(8):
                tile = ftiles[(0, t)]
                for u in range(2):
                    pr = 2 * t + u
                    eng = L0_ASSIGN[pr]
                    dst = sums0[:, pr:pr + 1]
                    if eng == "A":
                        nc.scalar.activation(scr0[:, :], tile[:, u, :],
                                             ACTF.Copy, accum_out=dst)
                    elif eng == "V":
                        nc.vector.tensor_reduce(dst, tile[:, u, :],
                                                axis=AX.X, op=ALU.add)
                    else:
                        h = hvp.tile([128, HW // 2], F16, tag="h0",
                                     name=f"h0_{pr}")
                        nc.gpsimd.tensor_tensor(
                            h[:], tile[:, u, 0:HW // 2],
                            tile[:, u, HW // 2:HW], op=ALU.add)
                        if eng == "GV":
                            nc.vector.tensor_reduce(dst, h[:], axis=AX.X,
                                                    op=ALU.add)
                        else:
                            nc.scalar.activation(scr0b[:, :], h[:],
                                                 ACTF.Copy, accum_out=dst)
            # qT0 [64, 16, 2]: col (u, h) = sample 2u+h <- sums0[64h:, u]
            for h in range(2):
                nc.scalar.mul(qT0[:, :, h], sums0[64 * h:64 * h + 64, :],
                              2.0 / HW)

            # L1
            C, HW, _ = LAYERS[1]
            for t in range(8):
                tile = ftiles[(1, t)]
                eng = L1_ASSIGN[t]
                dst = sums1[:, 4 * t:4 * t + 4]
                if eng == "V":
                    nc.vector.tensor_reduce(dst, tile[:, :, 0, :],
                                            axis=AX.X, op=ALU.add)
                elif eng == "A":
                    for g in range(4):
                        nc.scalar.activation(scr1[:, :], tile[:, g, 0, :],
                                             ACTF.Copy,
                                             accum_out=dst[:, g:g + 1])
                else:
                    h = hvp.tile([128, 4, HW // 2], F16, tag="h1",
                                 name=f"h1_{t}")
                    nc.gpsimd.tensor_tensor(h[:], tile[:, :, 0, 0:HW // 2],
                                            tile[:, :, 0, HW // 2:HW],
                                            op=ALU.add)
                    nc.vector.tensor_reduce(dst, h[:], axis=AX.X, op=ALU.add)
            nc.scalar.mul(qT1[:], sums1[:], 2.0 / HW)

            # L2 / L3
            for l in (2, 3):
                C, HW, cpp = LAYERS[l]
                assign = L2_ASSIGN if l == 2 else L3_ASSIGN
                sums = sums2 if l == 2 else sums3
                for t in range(8):
                    tile = ftiles[(l, t)]
                    eng = assign[t]
                    dst = sums[:, 4 * t:4 * t + 4, :]
                    if eng == "V":
                        nc.vector.tensor_reduce(dst, tile[:], axis=AX.X,
                                                op=ALU.add)
                    else:
                        hw2 = HW // 2  # 98 | 24 (49 odd: fold tail col below)
                        if l == 2:
                            h = hvp.tile([128, 4, cpp, hw2], F16,
                                         tag=f"h{l}", name=f"h{l}_{t}")
                            nc.gpsimd.tensor_tensor(
                                h[:], tile[:, :, :, 0:hw2],
                                tile[:, :, :, hw2:HW], op=ALU.add)
                            nc.vector.tensor_reduce(dst, h[:], axis=AX.X,
                                                    op=ALU.add)
                        else:
                            # 49 = 24 + 24 + 1: halve 48, add leftover col
                            h = hvp.tile([128, 4, cpp, hw2 + 1], F16,
                                         tag=f"h{l}", name=f"h{l}_{t}")
                            nc.gpsimd.tensor_tensor(
                                h[:, :, :, 0:hw2], tile[:, :, :, 0:hw2],
                                tile[:, :, :, hw2:2 * hw2], op=ALU.add)
                            nc.gpsimd.tensor_copy(h[:, :, :, hw2:hw2 + 1],
                                                  tile[:, :, :, 2 * hw2:HW])
                            nc.vector.tensor_reduce(dst, h[:], axis=AX.X,
                                                    op=ALU.add)
                # qT chunks: channel c = cpp*p + j
                for j in range(cpp):
                    nc.scalar.mul(qT23[(l, j)][:, 0:32] if l == 2
                                  else qT23[(l, j)][:, 32:64],
                                  sums[:, :, j], 2.0 / HW)

            # ---------------- distances into PSUM big [128, 2000]
            big = pbig.tile([128, R], F32, tag="big", name="big")
            for c0, n in RCHUNKS:
                cs = slice(c0, c0 + n)
                # quadrant 0: L0
                nc.tensor.matmul(big[0:32, cs], qT0[:], rt[(0, 0)][:, cs],
                                 start=True, stop=False)
                nc.tensor.matmul(big[0:32, cs], neg_full[0:64, 0:32],
                                 sq[(0, 0)][:, cs], start=False, stop=True)
                # quadrant 1: L1
                nc.tensor.matmul(big[32:64, cs], qT1[:], rt[(1, 0)][:, cs],
                                 start=True, stop=False)
                nc.tensor.matmul(big[32:64, cs], neg_full[:, 0:32],
                                 sq[(1, 0)][:, cs], start=False, stop=True)
                # quadrant 2: L2 (cols 0:32) + L3 (cols 32:64)
                first = True
                for l, nch in ((2, 2), (3, 4)):
                    mask = neg_l2 if l == 2 else neg_l3
                    for j in range(nch):
                        i = 128 * j
                        nc.tensor.matmul(big[64:128, cs], qT23[(l, j)][:],
                                         rt[(l, i)][:, cs],
                                         start=first, stop=False)
                        first = False
                        nc.tensor.matmul(
                            big[64:128, cs], mask[:], sq[(l, i)][:, cs],
                            start=False,
                            stop=(l == 3 and j == nch - 1))

            # ---------------- top-24 (split halves + merge)
            for hi, (h0, hn) in enumerate((HALF_A, HALF_B)):
                hs = slice(h0, h0 + hn)
                vb = 24 * hi
                nc.vector.max(vals48[:, vb:vb + 8], big[:, hs])
                nc.vector.match_replace(big[:, hs], vals48[:, vb:vb + 8],
                                        big[:, hs], NEG_BIG)
                nc.vector.max(vals48[:, vb + 8:vb + 16], big[:, hs])
                nc.vector.match_replace(big[:, hs],
                                        vals48[:, vb + 8:vb + 16],
                                        big[:, hs], NEG_BIG)
                nc.vector.max(vals48[:, vb + 16:vb + 24], big[:, hs])
            nc.vector.max(vals[:, 0:8], vals48[:])
            nc.vector.match_replace(vals48[:], vals[:, 0:8], vals48[:],
                                    NEG_BIG)
            nc.vector.max(vals[:, 8:16], vals48[:])
            nc.vector.match_replace(vals48[:], vals[:, 8:16], vals48[:],
                                    NEG_BIG)
            nc.vector.max(vals[:, 16:24], vals48[:])

            # ---------------- qn2 -> fixrow = -qn2 per (l, s) row
            qsq0 = pp.tile([64, 16, 2], F16, tag="qsq0", name="qsq0")
            qsq1 = pp.tile([128, B], F16, tag="qsq1", name="qsq1")
            nc.scalar.activation(qsq0[:], qT0[:], ACTF.Square, scale=0.5)
            nc.scalar.activation(qsq1[:], qT1[:], ACTF.Square, scale=0.5)
            qpsA = psml.tile([64, 1], F32, tag="qpsA", name="qpsA")
            nc.tensor.matmul(qpsA[0:32, :], qsq0[:], ones_col[0:64, 0:1],
                             start=True, stop=True)
            nc.tensor.matmul(qpsA[32:64, :], qsq1[:], ones_col[:, 0:1],
                             start=True, stop=True)
            qpsB = psml.tile([64, 1], F32, tag="qpsB", name="qpsB")
            first = True
            for l, nch in ((2, 2), (3, 4)):
                for j in range(nch):
                    qsq = pp.tile([128, 64], F16, tag=f"qsq{l}_{j}",
                                  name=f"qsq{l}_{j}")
                    nc.scalar.activation(qsq[:], qT23[(l, j)][:],
                                         ACTF.Square, scale=0.5)
                    nc.tensor.matmul(qpsB[:], qsq[:], ones_col[:, 0:1],
                                     start=first,
                                     stop=(l == 3 and j == nch - 1))
                    first = False
            for l in range(4):
                src = qpsA if l < 2 else qpsB
                r0 = 32 * (l % 2)
                nc.scalar.activation(fixrow[32 * l:32 * l + 32, :],
                                     src[r0:r0 + 32, :], ACTF.Copy,
                                     scale=-1.0)

            # ---------------- LID
            ln2 = pp.tile([128, 24], F32, tag="ln2", name="ln2")
            S = pp.tile([128, 1], F32, tag="S", name="S")
            denom = pp.tile([128, 1], F32, tag="denom", name="denom")
            lid = pp.tile([128, 1], F32, tag="lid", name="lid")
            nc.vector.tensor_scalar(vals[:], vals[:], fixrow[:], None,
                                    op0=ALU.add)
            nc.vector.tensor_scalar_min(vals[:], vals[:], -1e-30)
            nc.scalar.activation(ln2[:], vals[:], ACTF.Ln, scale=-1.0)
            nc.vector.tensor_reduce(S[:], ln2[:, 1:21], axis=AX.X,
                                    op=ALU.add)
            nc.vector.tensor_scalar(denom[:], ln2[:, 20:21], -20.0, S[:],
                                    op0=ALU.mult, op1=ALU.add)
            nc.vector.reciprocal(lid[:], denom[:])
            nc.vector.tensor_scalar_mul(lid[:], lid[:], -2.0 * K)

            # ---------------- regression + sigmoid
            lid4 = pp.tile([B, 4], F32, tag="lid4", name="lid4")
            for l in range(4):
                nc.vector.tensor_copy(lid4[:, l:l + 1],
                                      lid[32 * l:32 * l + 32, :])
            wps = psml.tile([B, 5], F32, tag="wps", name="wps")
            nc.tensor.matmul(wps[:], ones_row[:], wb_sb[:],
                             start=True, stop=True)
            wbc = pp.tile([B, 5], F32, tag="wbc", name="wbc")
            nc.scalar.copy(wbc[:], wps[:])
            prod = pp.tile([B, 4], F32, tag="prod", name="prod")
            nc.vector.tensor_tensor(prod[:], lid4[:], wbc[:, 0:4],
                                    op=ALU.mult)
            ssum = pp.tile([B, 1], F32, tag="ssum", name="ssum")
            nc.vector.tensor_reduce(ssum[:], prod[:], axis=AX.X, op=ALU.add)
            res = pp.tile([B, 1], F32, tag="res", name="res")
            nc.scalar.activation(res[:], ssum[:], ACTF.Sigmoid,
                                 bias=wbc[:, 4:5])
            nc.sync.dma_start(out=out[:], in_=res[:])

    nc.compile()
    return nc


_NC = None


def _get_nc():
    global _NC
    if _NC is None:
        _NC = build_nc()
    return _NC


def _perm(cpp, C):
    return [cpp * p + j for j in range(cpp) for p in range(C // cpp)]


def run(trace=False, **inputs):
    nc = _get_nc()
    feats = [np.asarray(inputs[f"feat{l}"], dtype=np.float32)
             for l in range(4)]
    refTs = []
    for l, (C, HW, cpp) in enumerate(LAYERS):
        rT = np.asarray(inputs[f"ref{l}"], dtype=np.float32).T  # [C, R]
        if cpp > 1:
            rT = rT[_perm(cpp, C)]
        refTs.append(np.ascontiguousarray(rT).astype(np.float16))
    regw = np.asarray(inputs["reg_w"], dtype=np.float32).reshape(1, 4)
    regb = np.asarray(inputs["reg_b"], dtype=np.float32).reshape(1, 1)
    assert int(inputs.get("k", K)) == K

    in_maps = []
    for c in range(N_CORES):
        m = {}
        for l, (C, HW, _) in enumerate(LAYERS):
            m[f"feat{l}"] = np.ascontiguousarray(
                feats[l][c * B:(c + 1) * B].reshape(B, C, HW)).astype(
                    ml_dtypes.float8_e4m3)
            m[f"refT{l}"] = refTs[l]
        m["regw"] = regw
        m["regb"] = regb
        in_maps.append(m)

    res = run_bass_kernel_spmd(nc, in_maps, core_ids=list(range(N_CORES)),
                               trace=trace)
    full = np.empty((N_CORES * B,), dtype=np.float32)
    for c in range(N_CORES):
        full[c * B:(c + 1) * B] = res.results[c]["out"][:, 0]
    return full, res


def kernel(**inputs):
    return run(trace=False, **inputs)[0]


# revision 7
# speedup vs baseline: 1.6009x; 1.1495x over previous
"""LID detector kernel for Trainium2 (8 NeuronCores, data-parallel over batch).

Per core (batch shard of 32 samples):
  - features arrive as fp8(e4m3); mean-pool over space on DVE/ACT/GPSIMD
    into f32 sums, scaled into fp16 qT tiles (holding 2*q)
  - refs arrive as fp16 [C, R] (host-transposed; L2/L3 rows permuted to match
    the on-chip channel interleave — distances are invariant to a shared
    channel permutation of q and ref)
  - big[row,(l,s), ref] = 2q.r - ||r||^2 accumulated by fp16 PE matmuls into
    one [128, 2000] PSUM tile: dist matmuls (lhsT=qT) plus fold matmuls
    (lhsT=-1s, rhs=sq=rt^2).  L2+L3 share the base-64 quadrant with
    zero-padded lhsT column blocks (matmul PSUM base partition must be
    0/32/64).
  - top-24 via DVE max8/match_replace straight on PSUM, split into two
    column halves + a 48-wide merge so half A overlaps half B's matmuls
  - vals += -qn2 per row -> -d2; LID = -2k/(sum ln d2_i - 20 ln d2_20);
    4->1 regression; sigmoid
"""

import sys

for _p in ("/opt/trn_rl_repo", "/root/.axon_site/_ro/trn_rl_repo"):
    if _p not in sys.path:
        sys.path.append(_p)

import ml_dtypes
import numpy as np

import concourse.mybir as mybir
from concourse import bass, bacc
from concourse.tile import TileContext
from concourse.bass_utils import run_bass_kernel_spmd

F32 = mybir.dt.float32
F16 = mybir.dt.float16
F8 = mybir.dt.float8e4
N_CORES = 8
B = 32
R = 2000
K = 20
NEG_BIG = -3.0e38
ALU = mybir.AluOpType
ACTF = mybir.ActivationFunctionType
AX = mybir.AxisListType

# (C, HW, channels-per-partition)
LAYERS = [(64, 3136, 1), (128, 784, 1), (256, 196, 2), (512, 49, 4)]

# --- engine assignment knobs -----------------------------------------------
# L0: 16 pairs (2 samples x 64ch on 128 partitions, 3136 free cols each).
#   'A' ACT accum | 'V' DVE reduce | 'GV'/'GA' gpsimd halve + DVE/ACT tail
L0_ASSIGN = ["GV", "A", "GV", "A", "GV", "A", "GV", "A",
             "GV", "A", "GV", "A", "GV", "A", "GV", "A"]
# L1: 8 tiles [128, 4, 784]. 'V' | 'A' (4 ops) | 'GV'
L1_ASSIGN = ["GV", "A", "V", "GV", "A", "V", "GV", "A"]
# L2: 8 tiles [128, 4, 2, 196]. 'V' | 'GV'
L2_ASSIGN = ["GV", "V", "GV", "V", "GV", "V", "GV", "V"]
# L3: 8 tiles [128, 4, 4, 49]. 'V' | 'GV'
L3_ASSIGN = ["V", "V", "V", "V", "V", "V", "V", "V"]
# squares, keyed (layer, chunk_offset): 'A' | 'V' | 'G'
SQ_ASSIGN = {(0, 0): "A", (1, 0): "A", (2, 0): "A", (2, 128): "A",
             (3, 0): "G", (3, 128): "G", (3, 256): "A", (3, 384): "A"}

RCHUNKS = [(0, 512), (512, 512), (1024, 512), (1536, 464)]
HALF_A = (0, 1024)
HALF_B = (1024, 976)


def build_nc():
    nc = bacc.Bacc("TRN2", target_bir_lowering=False, debug=False,
                   num_devices=N_CORES)

    feats = [nc.dram_tensor(f"feat{l}", [B, C, HW], F8, kind="ExternalInput")
             for l, (C, HW, _) in enumerate(LAYERS)]
    refTs = [nc.dram_tensor(f"refT{l}", [C, R], F16, kind="ExternalInput")
             for l, (C, _, _) in enumerate(LAYERS)]
    regw = nc.dram_tensor("regw", [1, 4], F32, kind="ExternalInput")
    regb = nc.dram_tensor("regb", [1, 1], F32, kind="ExternalInput")
    out = nc.dram_tensor("out", [B, 1], F32, kind="ExternalOutput")

    with TileContext(nc) as tc:
        with (
            tc.tile_pool(name="persist", bufs=1) as pp,
            tc.tile_pool(name="ft0", bufs=3) as fp0,
            tc.tile_pool(name="ft1", bufs=3) as fp1,
            tc.tile_pool(name="ft23", bufs=3) as fp23,
            tc.tile_pool(name="half", bufs=3) as hvp,
            tc.tile_pool(name="pbig", bufs=1,
                         space=bass.MemorySpace.PSUM) as pbig,
            tc.tile_pool(name="psmall", bufs=1,
                         space=bass.MemorySpace.PSUM) as psml,
        ):
            # ---------------- persistent tiles
            rt = {}
            sq = {}
            for l, (C, _, _) in enumerate(LAYERS):
                for i in range(0, C, 128):
                    Cc = min(128, C - i)
                    rt[(l, i)] = pp.tile([Cc, R], F16, tag=f"rt{l}_{i}",
                                         name=f"rt{l}_{i}")
                    sq[(l, i)] = pp.tile([Cc, R], F16, tag=f"sq{l}_{i}",
                                         name=f"sq{l}_{i}")
            sums0 = pp.tile([128, 16], F32, tag="sums0", name="sums0")
            sums1 = pp.tile([128, B], F32, tag="sums1", name="sums1")
            sums2 = pp.tile([128, B, 2], F32, tag="sums2", name="sums2")
            sums3 = pp.tile([128, B, 4], F32, tag="sums3", name="sums3")
            qT0 = pp.tile([64, 16, 2], F16, tag="qT0", name="qT0")
            qT1 = pp.tile([128, B], F16, tag="qT1", name="qT1")
            qT23 = {}
            for l, nch in ((2, 2), (3, 4)):
                for j in range(nch):
                    qT23[(l, j)] = pp.tile([128, 64], F16, tag=f"qT{l}_{j}",
                                           name=f"qT{l}_{j}")
            ones_col = pp.tile([128, 1], F16, tag="ones_col", name="ones_col")
            ones_row = pp.tile([1, B], F32, tag="ones_row", name="ones_row")
            # fold lhsT masks: row usage via column slices
            neg_full = pp.tile([128, 64], F16, tag="neg_full", name="neg_full")
            neg_l2 = pp.tile([128, 64], F16, tag="neg_l2", name="neg_l2")
            neg_l3 = pp.tile([128, 64], F16, tag="neg_l3", name="neg_l3")
            wb_sb = pp.tile([1, 5], F32, tag="wb_sb", name="wb_sb")
            vals48 = pp.tile([128, 48], F32, tag="vals48", name="vals48")
            vals = pp.tile([128, 24], F32, tag="vals", name="vals")
            fixrow = pp.tile([128, 1], F32, tag="fixrow", name="fixrow")
            scr0 = pp.tile([128, 3136], F8, tag="scr0", name="scr0")
            scr0b = pp.tile([128, 1568], F16, tag="scr0b", name="scr0b")
            scr1 = pp.tile([128, 784], F8, tag="scr1", name="scr1")

            nc.vector.memset(ones_col[:], 1.0)
            nc.vector.memset(ones_row[:], 1.0)
            nc.vector.memset(neg_full[:], -1.0)
            nc.vector.memset(neg_l2[:, 0:32], -1.0)
            nc.vector.memset(neg_l2[:, 32:64], 0.0)
            nc.vector.memset(neg_l3[:, 0:32], 0.0)
            nc.vector.memset(neg_l3[:, 32:64], -1.0)
            for l, nch in ((2, 2), (3, 4)):
                zs = slice(32, 64) if l == 2 else slice(0, 32)
                for j in range(nch):
                    nc.vector.memset(qT23[(l, j)][:, zs], 0.0)
            nc.sync.dma_start(out=wb_sb[0:1, 0:4], in_=regw[:])
            nc.sync.dma_start(out=wb_sb[0:1, 4:5], in_=regb[:])

            # ---------------- DMAs: L2/L3 + refs first, then L0, then L1
            # (small layers pool early so their long distance-matmul chains
            # overlap L0/L1 pooling; L0/L1 dists at the end are short)
            ref_order = [(3, 0), (3, 128), (3, 256), (3, 384),
                         (2, 0), (2, 128), (1, 0), (0, 0)]
            ftiles = {}

            def dma_feat(l, t):
                C, HW, cpp = LAYERS[l]
                nS = 2 if l == 0 else 4
                if l == 0:
                    tile = fp0.tile([128, nS, HW], F8, tag="f0",
                                    name=f"f0_{t}")
                    src = bass.AP(feats[0], 4 * t * C * HW,
                                  [[HW, 128], [2 * C * HW, nS], [1, HW]])
                else:
                    fpool = fp1 if l == 1 else fp23
                    tile = fpool.tile([128, nS, cpp, HW], F8, tag=f"f{l}",
                                      name=f"f{l}_{t}")
                    src = bass.AP(feats[l], nS * t * C * HW,
                                  [[cpp * HW, 128], [C * HW, nS],
                                   [HW, cpp], [1, HW]])
                nc.sync.dma_start(out=tile[:], in_=src)
                ftiles[(l, t)] = tile

            for t in range(8):
                dma_feat(2, t)
                li = ref_order[t]
                nc.sync.dma_start(
                    out=rt[li][:],
                    in_=refTs[li[0]][li[1]:li[1] + rt[li].shape[0], :])
            for t in range(8):
                dma_feat(3, t)
            for t in range(8):
                dma_feat(0, t)
            for t in range(8):
                dma_feat(1, t)

            # ---------------- squares (sq = rt^2, fp16), ref-arrival order
            for (l, i) in ref_order:
                eng = SQ_ASSIGN[(l, i)]
                s_, r_ = sq[(l, i)], rt[(l, i)]
                if eng == "A":
                    nc.scalar.square(s_[:], r_[:])
                elif eng == "V":
                    nc.vector.tensor_tensor(s_[:], r_[:], r_[:], op=ALU.mult)
                else:
                    nc.gpsimd.tensor_tensor(s_[:], r_[:], r_[:], op=ALU.mult)

            # ---------------- fold matmuls first: big = -sum_c r_c^2
            # (depend only on refs/squares -> PE busy during pooling)
            big = pbig.tile([128, R], F32, tag="big", name="big")
            for c0, n in RCHUNKS:
                cs = slice(c0, c0 + n)
                for j in range(4):
                    i = 128 * j
                    nc.tensor.matmul(big[64:128, cs], neg_l3[:],
                                     sq[(3, i)][:, cs],
                                     start=(j == 0), stop=False)
                for j in range(2):
                    i = 128 * j
                    nc.tensor.matmul(big[64:128, cs], neg_l2[:],
                                     sq[(2, i)][:, cs],
                                     start=False, stop=False)
                nc.tensor.matmul(big[32:64, cs], neg_full[:, 0:32],
                                 sq[(1, 0)][:, cs], start=True, stop=False)
                nc.tensor.matmul(big[0:32, cs], neg_full[0:64, 0:32],
                                 sq[(0, 0)][:, cs], start=True, stop=False)

            # ---------------- pooling: L2, L3 first, then L0, then L1
            # L2 / L3
            for l in (2, 3):
                C, HW, cpp = LAYERS[l]
                assign = L2_ASSIGN if l == 2 else L3_ASSIGN
                sums = sums2 if l == 2 else sums3
                for t in range(8):
                    tile = ftiles[(l, t)]
                    eng = assign[t]
                    dst = sums[:, 4 * t:4 * t + 4, :]
                    if eng == "V":
                        nc.vector.tensor_reduce(dst, tile[:], axis=AX.X,
                                                op=ALU.add)
                    else:
                        hw2 = HW // 2  # 98 | 24 (49 odd: copy tail col)
                        if l == 2:
                            h = hvp.tile([128, 4, cpp, hw2], F16,
                                         tag=f"h{l}", name=f"h{l}_{t}")
                            nc.gpsimd.tensor_tensor(
                                h[:], tile[:, :, :, 0:hw2],
                                tile[:, :, :, hw2:HW], op=ALU.add)
                            nc.vector.tensor_reduce(dst, h[:], axis=AX.X,
                                                    op=ALU.add)
                        else:
                            h = hvp.tile([128, 4, cpp, hw2 + 1], F16,
                                         tag=f"h{l}", name=f"h{l}_{t}")
                            nc.gpsimd.tensor_tensor(
                                h[:, :, :, 0:hw2], tile[:, :, :, 0:hw2],
                                tile[:, :, :, hw2:2 * hw2], op=ALU.add)
                            nc.gpsimd.tensor_copy(h[:, :, :, hw2:hw2 + 1],
                                                  tile[:, :, :, 2 * hw2:HW])
                            nc.vector.tensor_reduce(dst, h[:], axis=AX.X,
                                                    op=ALU.add)
                for j in range(cpp):
                    nc.scalar.mul(qT23[(l, j)][:, 0:32] if l == 2
                                  else qT23[(l, j)][:, 32:64],
                                  sums[:, :, j], 2.0 / HW)

            # ---------------- quadrant-2 distances (overlap L0/L1 pooling)
            def dist_q2(c0, n, stop):
                cs = slice(c0, c0 + n)
                for l, nch in ((2, 2), (3, 4)):
                    for j in range(nch):
                        nc.tensor.matmul(
                            big[64:128, cs], qT23[(l, j)][:],
                            rt[(l, 128 * j)][:, cs], start=False,
                            stop=(stop and l == 3 and j == nch - 1))

            for c0, n in RCHUNKS:
                dist_q2(c0, n, True)

            # L0: pairs
            C, HW, _ = LAYERS[0]
            for t in range(8):
                tile = ftiles[(0, t)]
                for u in range(2):
                    pr = 2 * t + u
                    eng = L0_ASSIGN[pr]
                    dst = sums0[:, pr:pr + 1]
                    if eng == "A":
                        nc.scalar.activation(scr0[:, :], tile[:, u, :],
                                             ACTF.Copy, accum_out=dst)
                    elif eng == "V":
                        nc.vector.tensor_reduce(dst, tile[:, u, :],
                                                axis=AX.X, op=ALU.add)
                    else:
                        h = hvp.tile([128, HW // 2], F16, tag="h0",
                                     name=f"h0_{pr}")
                        nc.gpsimd.tensor_tensor(
                            h[:], tile[:, u, 0:HW // 2],
                            tile[:, u, HW // 2:HW], op=ALU.add)
                        if eng == "GV":
                            nc.vector.tensor_reduce(dst, h[:], axis=AX.X,
                                                    op=ALU.add)
                        else:
                            nc.scalar.activation(scr0b[:, :], h[:],
                                                 ACTF.Copy, accum_out=dst)
            # qT0 [64, 16, 2]: col (u, h) = sample 2u+h <- sums0[64h:, u]
            for h in range(2):
                nc.scalar.mul(qT0[:, :, h], sums0[64 * h:64 * h + 64, :],
                              2.0 / HW)
            for c0, n in RCHUNKS:
                cs = slice(c0, c0 + n)
                nc.tensor.matmul(big[0:32, cs], qT0[:], rt[(0, 0)][:, cs],
                                 start=False, stop=True)

            # L1
            C, HW, _ = LAYERS[1]
            for t in range(8):
                tile = ftiles[(1, t)]
                eng = L1_ASSIGN[t]
                dst = sums1[:, 4 * t:4 * t + 4]
                if eng == "V":
                    nc.vector.tensor_reduce(dst, tile[:, :, 0, :],
                                            axis=AX.X, op=ALU.add)
                elif eng == "A":
                    for g in range(4):
                        nc.scalar.activation(scr1[:, :], tile[:, g, 0, :],
                                             ACTF.Copy,
                                             accum_out=dst[:, g:g + 1])
                else:
                    h = hvp.tile([128, 4, HW // 2], F16, tag="h1",
                                 name=f"h1_{t}")
                    nc.gpsimd.tensor_tensor(h[:], tile[:, :, 0, 0:HW // 2],
                                            tile[:, :, 0, HW // 2:HW],
                                            op=ALU.add)
                    nc.vector.tensor_reduce(dst, h[:], axis=AX.X, op=ALU.add)
            nc.scalar.mul(qT1[:], sums1[:], 2.0 / HW)
            for c0, n in RCHUNKS:
                cs = slice(c0, c0 + n)
                nc.tensor.matmul(big[32:64, cs], qT1[:], rt[(1, 0)][:, cs],
                                 start=False, stop=True)

            # ---------------- top-24 (split halves + merge)
            for hi, (h0, hn) in enumerate((HALF_A, HALF_B)):
                hs = slice(h0, h0 + hn)
                vb = 24 * hi
                nc.vector.max(vals48[:, vb:vb + 8], big[:, hs])
                nc.vector.match_replace(big[:, hs], vals48[:, vb:vb + 8],
                                        big[:, hs], NEG_BIG)
                nc.vector.max(vals48[:, vb + 8:vb + 16], big[:, hs])
                nc.vector.match_replace(big[:, hs],
                                        vals48[:, vb + 8:vb + 16],
                                        big[:, hs], NEG_BIG)
                nc.vector.max(vals48[:, vb + 16:vb + 24], big[:, hs])
            nc.vector.max(vals[:, 0:8], vals48[:])
            nc.vector.match_replace(vals48[:], vals[:, 0:8], vals48[:],
                                    NEG_BIG)
            nc.vector.max(vals[:, 8:16], vals48[:])
            nc.vector.match_replace(vals48[:], vals[:, 8:16], vals48[:],
                                    NEG_BIG)
            nc.vector.max(vals[:, 16:24], vals48[:])

            # ---------------- qn2 -> fixrow = -qn2 per (l, s) row
            qsq0 = pp.tile([64, 16, 2], F16, tag="qsq0", name="qsq0")
            qsq1 = pp.tile([128, B], F16, tag="qsq1", name="qsq1")
            nc.scalar.activation(qsq0[:], qT0[:], ACTF.Square, scale=0.5)
            nc.scalar.activation(qsq1[:], qT1[:], ACTF.Square, scale=0.5)
            qpsA = psml.tile([64, 1], F32, tag="qpsA", name="qpsA")
            nc.tensor.matmul(qpsA[0:32, :], qsq0[:], ones_col[0:64, 0:1],
                             start=True, stop=True)
            nc.tensor.matmul(qpsA[32:64, :], qsq1[:], ones_col[:, 0:1],
                             start=True, stop=True)
            qpsB = psml.tile([64, 1], F32, tag="qpsB", name="qpsB")
            first = True
            for l, nch in ((2, 2), (3, 4)):
                for j in range(nch):
                    qsq = pp.tile([128, 64], F16, tag=f"qsq{l}_{j}",
                                  name=f"qsq{l}_{j}")
                    nc.scalar.activation(qsq[:], qT23[(l, j)][:],
                                         ACTF.Square, scale=0.5)
                    nc.tensor.matmul(qpsB[:], qsq[:], ones_col[:, 0:1],
                                     start=first,
                                     stop=(l == 3 and j == nch - 1))
                    first = False
            for l in range(4):
                src = qpsA if l < 2 else qpsB
                r0 = 32 * (l % 2)
                nc.scalar.activation(fixrow[32 * l:32 * l + 32, :],
                                     src[r0:r0 + 32, :], ACTF.Copy,
                                     scale=-1.0)

            # ---------------- LID
            ln2 = pp.tile([128, 24], F32, tag="ln2", name="ln2")
            S = pp.tile([128, 1], F32, tag="S", name="S")
            denom = pp.tile([128, 1], F32, tag="denom", name="denom")
            lid = pp.tile([128, 1], F32, tag="lid", name="lid")
            nc.vector.tensor_scalar(vals[:], vals[:], fixrow[:], None,
                                    op0=ALU.add)
            nc.vector.tensor_scalar_min(vals[:], vals[:], -1e-30)
            nc.scalar.activation(ln2[:], vals[:], ACTF.Ln, scale=-1.0)
            nc.vector.tensor_reduce(S[:], ln2[:, 1:21], axis=AX.X,
                                    op=ALU.add)
            nc.vector.tensor_scalar(denom[:], ln2[:, 20:21], -20.0, S[:],
                                    op0=ALU.mult, op1=ALU.add)
            nc.vector.reciprocal(lid[:], denom[:])
            nc.vector.tensor_scalar_mul(lid[:], lid[:], -2.0 * K)

            # ---------------- regression + sigmoid
            lid4 = pp.tile([B, 4], F32, tag="lid4", name="lid4")
            for l in range(4):
                nc.vector.tensor_copy(lid4[:, l:l + 1],
                                      lid[32 * l:32 * l + 32, :])
            wps = psml.tile([B, 5], F32, tag="wps", name="wps")
            nc.tensor.matmul(wps[:], ones_row[:], wb_sb[:],
                             start=True, stop=True)
            wbc = pp.tile([B, 5], F32, tag="wbc", name="wbc")
            nc.scalar.copy(wbc[:], wps[:])
            prod = pp.tile([B, 4], F32, tag="prod", name="prod")
            nc.vector.tensor_tensor(prod[:], lid4[:], wbc[:, 0:4],
                                    op=ALU.mult)
            ssum = pp.tile([B, 1], F32, tag="ssum", name="ssum")
            nc.vector.tensor_reduce(ssum[:], prod[:], axis=AX.X, op=ALU.add)
            res = pp.tile([B, 1], F32, tag="res", name="res")
            nc.scalar.activation(res[:], ssum[:], ACTF.Sigmoid,
                                 bias=wbc[:, 4:5])
            nc.sync.dma_start(out=out[:], in_=res[:])

    nc.compile()
    return nc


_NC = None


def _get_nc():
    global _NC
    if _NC is None:
        _NC = build_nc()
    return _NC


def _perm(cpp, C):
    return [cpp * p + j for j in range(cpp) for p in range(C // cpp)]


def run(trace=False, **inputs):
    nc = _get_nc()
    feats = [np.asarray(inputs[f"feat{l}"], dtype=np.float32)
             for l in range(4)]
    refTs = []
    for l, (C, HW, cpp) in enumerate(LAYERS):
        rT = np.asarray(inputs[f"ref{l}"], dtype=np.float32).T  # [C, R]
        if cpp > 1:
            rT = rT[_perm(cpp, C)]
        refTs.append(np.ascontiguousarray(rT).astype(np.float16))
    regw = np.asarray(inputs["reg_w"], dtype=np.float32).reshape(1, 4)
    regb = np.asarray(inputs["reg_b"], dtype=np.float32).reshape(1, 1)
    assert int(inputs.get("k", K)) == K

    in_maps = []
    for c in range(N_CORES):
        m = {}
        for l, (C, HW, _) in enumerate(LAYERS):
            m[f"feat{l}"] = np.ascontiguousarray(
                feats[l][c * B:(c + 1) * B].reshape(B, C, HW)).astype(
                    ml_dtypes.float8_e4m3)
            m[f"refT{l}"] = refTs[l]
        m["regw"] = regw
        m["regb"] = regb
        in_maps.append(m)

    res = run_bass_kernel_spmd(nc, in_maps, core_ids=list(range(N_CORES)),
                               trace=trace)
    full = np.empty((N_CORES * B,), dtype=np.float32)
    for c in range(N_CORES):
        full[c * B:(c + 1) * B] = res.results[c]["out"][:, 0]
    return full, res


def kernel(**inputs):
    return run(trace=False, **inputs)[0]
